# revision 22
# baseline (speedup 1.0000x reference)
"""GCN 3-layer kernel for Trainium2, 8-core SPMD — v7.

Math (per layer, PyG GCN convention, factorized):
    deg[d]  = indegree(d) + 1;  dinv = deg^-1/2
    y       = dinv[:,None] * (h @ W)                    (message table)
    agg[d]  = sum_{e: dst[e]=d} y[src[e]]               (edges only)
    h_next  = dinv[:,None] * agg + (dinv*y + b)         (self-loop + bias
              folded into a precomputed  y2b = dinv^2*(h@W) + b  table)

Distribution: destination-sharded across 8 cores (6272 nodes/core, padded
50176 total).  Per layer the y table is replicated to every core as FOUR
AllGathers over window quarters (0-12 / 13-24 / 25-36 / 37-48).  Each
quarter's AG is issued as soon as the producing epilogues of the previous
layer finish (after groups 2 / 4 / 7 / 9), so all but the last ~15us of
the exchange hides under the previous layer's gathers.  Quartering also
keeps every gather index below 2^15 (int16).

Each core gathers message rows for its incoming edges with dma_gather.
Q7 descriptor generation (~7 ns/row per Q7 core-pair) is the pipeline
bottleneck, so the four quarter-streams of each 5-window chunk run on the
four SWDGE queues = four Q7 core-pairs concurrently.  Messages are
scatter-added with one-hot matmuls on the PE (PSUM accumulation per
128-dst window); the one-hot S for a whole window (all four streams
concatenated) is built with a single broadcast tensor_tensor is_equal.
The next layer's y/y2b rows are produced in the same per-window epilogue
(transpose + matmul fused), feeding the next AG chunk.
"""

import numpy as np
import ml_dtypes

N_NODES = 50000
N_CORES = 8
PER_CORE = 6272            # 49 * 128
N_PAD = PER_CORE * N_CORES # 50176
N_WIN = 49
QUARTERS = [(0, 25), (25, 49)]  # window ranges per table half
NQ = 2
F = 128                    # feature width (layer3 padded 64->128)
F_OUT = 64
GROUP_WINDOWS = 7          # windows per gather chunk
# AG for half q fires after this group index (group covers its windows)
AG_AFTER_GROUP = [3, 6]

BF16 = ml_dtypes.bfloat16


def _wrap_idx16(idx: np.ndarray) -> np.ndarray:
    """Wrap a flat int16 index stream into the [128, n/16] layout dma_gather
    expects (element i at [i%16, i//16], replicated across the 8 groups of
    16 partitions)."""
    n = len(idx)
    assert n % 128 == 0
    cols = n // 16
    out = np.empty((128, cols), np.int16)
    w = idx.reshape(cols, 16).T  # [16, cols]
    for g in range(8):
        out[g * 16:(g + 1) * 16, :] = w
    return out


def _preprocess(edge_index: np.ndarray):
    """Host-side graph prep: degree norm, dst-sharding, per-(window, quarter)
    edge streams, block padding shared across cores.  Self-loops are NOT
    materialized as edges (handled via the y2b table)."""
    src = edge_index[0].astype(np.int64)
    dst = edge_index[1].astype(np.int64)
    deg = np.bincount(dst, minlength=N_NODES).astype(np.float64) + 1.0
    dinv = (1.0 / np.sqrt(deg)).astype(np.float32)
    dinv_pad = np.ones(N_PAD, np.float32)
    dinv_pad[:N_NODES] = dinv

    core_of = dst // PER_CORE
    win_of = (dst % PER_CORE) // 128
    dloc_of = dst % 128

    src_core = src // PER_CORE
    src_off = src % PER_CORE
    src_win = src_off // 128
    q_of = np.searchsorted([q[1] for q in QUARTERS], src_win, side="right")
    rows_q = np.array([(q[1] - q[0]) * 128 for q in QUARTERS])
    w0_q = np.array([q[0] * 128 for q in QUARTERS])
    idx_val = src_core * rows_q[q_of] + (src_off - w0_q[q_of])

    order = np.lexsort((dst, q_of, win_of, core_of))
    core_s, win_s, dloc_s, q_s, iv_s = (
        core_of[order], win_of[order], dloc_of[order], q_of[order],
        idx_val[order])

    # per (core, window, quarter) counts -> shared block counts
    counts = np.zeros((N_CORES, N_WIN, NQ), np.int64)
    np.add.at(counts, (core_s, win_s, q_s), 1)
    blk = np.maximum(1, -(-counts.max(axis=0) // 128))  # [N_WIN, NQ]

    # per-quarter stream offsets: stream q holds its blocks window-major
    off = np.zeros((NQ, N_WIN + 1), np.int64)
    for q in range(NQ):
        off[q, 1:] = np.cumsum(blk[:, q] * 128)
    n_q = off[:, -1].astype(int)  # slots per stream

    idx_q = [np.zeros((N_CORES, int(n)), np.int16) for n in n_q]
    dl_q = [np.full((N_CORES, int(n)), 999.0, np.float32) for n in n_q]

    keys = (core_s * N_WIN + win_s) * NQ + q_s
    bounds = np.searchsorted(keys, np.arange(N_CORES * N_WIN * NQ + 1))
    for c in range(N_CORES):
        for w in range(N_WIN):
            for q in range(NQ):
                k = (c * N_WIN + w) * NQ + q
                sl = slice(bounds[k], bounds[k + 1])
                iv = iv_s[sl]; dl = dloc_s[sl]
                o = off[q, w]
                idx_q[q][c, o:o + len(iv)] = iv.astype(np.int16)
                dl_q[q][c, o:o + len(iv)] = dl

    # combined per-window dl (all quarters' blocks of window w contiguous),
    # matching the matmul consumption order
    blk_w = blk.sum(axis=1)             # blocks per window
    off_w = np.concatenate([[0], np.cumsum(blk_w)])  # block offsets
    n_blk = int(off_w[-1])
    dl_win = np.full((N_CORES, n_blk * 128), 999.0, np.float32)
    for c in range(N_CORES):
        for w in range(N_WIN):
            o = off_w[w] * 128
            for q in range(NQ):
                nbq = int(blk[w, q]) * 128
                dl_win[c, o:o + nbq] = dl_q[q][c, off[q, w]:off[q, w] + nbq]
                o += nbq

    return (dinv_pad, blk, off, idx_q, dl_win, blk_w, off_w)


def _build_and_run(inputs_np, dinv_pad, blk, off, idx_q, dl_win, blk_w, off_w,
                   trace=False, sim=False):
    import concourse.bacc as bacc
    import concourse.mybir as mybir
    from concourse.tile import TileContext
    from concourse import bass, bass_utils, library_config
    from concourse.masks import make_identity

    x = inputs_np["x"]
    Ws = [np.asarray(inputs_np[k], np.float32) for k in ("W1", "W2", "W3")]
    bs = [np.asarray(inputs_np[k], np.float32) for k in ("b1", "b2", "b3")]
    # pad W3/b3 to 128 output features
    W3p = np.zeros((F, F), np.float32); W3p[:, :F_OUT] = Ws[2]
    b3p = np.zeros(F, np.float32); b3p[:F_OUT] = bs[2]
    Ws[2], bs[2] = W3p, b3p

    n_q = [int(idx_q[q].shape[1]) for q in range(NQ)]
    n_blk = int(off_w[-1])
    G = GROUP_WINDOWS
    groups = [list(range(g, min(g + G, N_WIN))) for g in range(0, N_WIN, G)]
    cap_blk = max(int(off_w[g[-1] + 1] - off_w[g[0]]) for g in groups)
    # per (group, half) slot ranges, each split into two block-balanced subs
    def subsplit(q0, q1):
        mid = q0 + ((q1 - q0) // 256) * 128
        return ((q0, mid), (mid, q1))
    gr = [[subsplit(int(off[q, g[0]]), int(off[q, g[-1] + 1]))
           for q in range(NQ)] for g in groups]
    cap = [[max(r[q][i][1] - r[q][i][0] for r in gr) // 128 for i in range(2)]
           for q in range(NQ)]
    rows_q = [(q1 - q0) * 128 for q0, q1 in QUARTERS]

    nc = bacc.Bacc("TRN2", target_bir_lowering=False, debug=False,
                   num_devices=N_CORES, num_swdge_queues=4)
    dt = mybir.dt
    Alu = mybir.AluOpType
    Act = mybir.ActivationFunctionType

    # ---- kernel I/O -----------------------------------------------------
    t_xT = nc.dram_tensor("xT_own", [128, PER_CORE], dt.float32, kind="ExternalInput")
    t_W = [nc.dram_tensor(f"W{i+1}m", [F, F], dt.float32, kind="ExternalInput") for i in range(3)]
    t_b = [nc.dram_tensor(f"b{i+1}m", [128, F], dt.float32, kind="ExternalInput") for i in range(3)]
    t_dinv = nc.dram_tensor("dinv_own", [128, N_WIN], dt.float32, kind="ExternalInput")
    t_dinv2 = nc.dram_tensor("dinv2_own", [128, N_WIN], dt.float32, kind="ExternalInput")
    t_iota = nc.dram_tensor("iota", [128, 128], dt.bfloat16, kind="ExternalInput")
    t_iq = [nc.dram_tensor(f"idx_q{q}", [128, n_q[q] // 16], dt.int16,
                           kind="ExternalInput") for q in range(NQ)]
    t_dlw = nc.dram_tensor("dl_win", [128, n_blk], dt.bfloat16, kind="ExternalInput")
    t_out = nc.dram_tensor("h_out", [PER_CORE, F_OUT], dt.float32, kind="ExternalOutput")

    with TileContext(nc) as tc:
        nc.gpsimd.load_library(library_config.mlp)
        with tc.tile_pool(name="const", bufs=1) as cpool, \
             tc.tile_pool(name="state", bufs=1) as spool, \
             tc.tile_pool(name="gath", bufs=2) as gpool, \
             tc.tile_pool(name="sbld", bufs=6) as sbld, \
             tc.tile_pool(name="work", bufs=3) as wpool, \
             tc.tile_pool(name="stg", bufs=2) as stg, \
             tc.tile_pool(name="psA", bufs=4, space="PSUM") as psA, \
             tc.tile_pool(name="psT", bufs=2, space="PSUM") as psT, \
             tc.tile_pool(name="ps2", bufs=2, space="PSUM") as ps2p, \
             tc.tile_pool(name="dram", bufs=1, space="DRAM") as dpool:

            # ---- constants ----
            c_W = [cpool.tile([F, F], dt.float32, tag=f"W{i}", name=f"cW{i}") for i in range(3)]
            c_b = [cpool.tile([128, F], dt.float32, tag=f"b{i}", name=f"cb{i}") for i in range(3)]
            c_dinv = cpool.tile([128, N_WIN], dt.float32, tag="dinv", name="dinv")
            c_dinv2 = cpool.tile([128, N_WIN], dt.float32, tag="dinv2", name="dinv2")
            c_iota = cpool.tile([128, 128], dt.bfloat16, tag="iota", name="iota")
            c_iq = [cpool.tile([128, n_q[q] // 16], dt.int16, tag=f"iq{q}",
                               name=f"iq{q}") for q in range(NQ)]
            c_dlw = cpool.tile([128, n_blk], dt.bfloat16, tag="dlw", name="dlw")
            c_ident = cpool.tile([128, 128], dt.float32, tag="ident", name="ident")
            for i in range(3):
                nc.sync.dma_start(c_W[i][:], t_W[i][:])
                nc.sync.dma_start(c_b[i][:], t_b[i][:])
            nc.sync.dma_start(c_dinv[:], t_dinv[:])
            nc.sync.dma_start(c_dinv2[:], t_dinv2[:])
            nc.sync.dma_start(c_iota[:], t_iota[:])
            for q in range(NQ):
                nc.sync.dma_start(c_iq[q][:], t_iq[q][:])
            nc.sync.dma_start(c_dlw[:], t_dlw[:])
            make_identity(nc, c_ident[:])

            # ---- persistent state: y2b = dinv^2*(h@W) + b, two generations
            y2b = [spool.tile([128, N_WIN, F], dt.float32, tag="y2b_a", name="y2b_a"),
                   spool.tile([128, N_WIN, F], dt.float32, tag="y2b_b", name="y2b_b")]

            y_full = [[dpool.tile([N_CORES * rows_q[q], F], dt.bfloat16,
                                  addr_space="Shared", name=f"y_full{i}_{q}")
                       for q in range(NQ)] for i in range(3)]
            ag_in = [[dpool.tile([rows_q[q], F], dt.bfloat16, name=f"ag_in{i}_{q}")
                      for q in range(NQ)] for i in range(3)]

            def stage_y(ps, w, wi, layer_next, yst):
                """From PSUM ps = (h @ W_next) for window w: stage bf16 y row
                block and write fp32 y2b (self+bias) for the next layer."""
                nc.scalar.mul(yst[:, wi, :], ps[:], c_dinv[:, w:w + 1])
                nc.vector.scalar_tensor_tensor(
                    out=y2b[layer_next % 2][:, w, :], in0=ps[:],
                    scalar=c_dinv2[:, w:w + 1], in1=c_b[layer_next][:],
                    op0=Alu.mult, op1=Alu.add)

            def flush_y(g, layer_next, yst):
                """DMA the staged bf16 y rows of group g to the AG inputs.
                A group may straddle a quarter boundary."""
                w0, w1 = g[0], g[-1] + 1
                s = w0
                while s < w1:
                    q = next(i for i, (a, b) in enumerate(QUARTERS)
                             if a <= s < b)
                    e = min(w1, QUARTERS[q][1])
                    dst = ag_in[layer_next][q][
                        (s - QUARTERS[q][0]) * 128:(e - QUARTERS[q][0]) * 128, :]
                    nc.sync.dma_start(dst.rearrange("(t p) f -> p t f", p=128),
                                      yst[:, s - w0:e - w0, :])
                    s = e

            def ag_quarter(layer, q):
                nc.gpsimd.collective_compute(
                    "AllGather", Alu.bypass,
                    replica_groups=[list(range(N_CORES))],
                    ins=[ag_in[layer][q].opt()], outs=[y_full[layer][q].opt()])

            # ---- layer 0 phase A: y1 = dinv*(x@W1) ----
            with tc.tile_pool(name="xp", bufs=1) as xpool:
                xT = xpool.tile([128, PER_CORE], dt.float32, tag="xT", name="xT")
                nc.sync.dma_start(xT[:], t_xT[:])
                nags = 0
                for gi, g in enumerate(groups):
                    yst = stg.tile([128, G, F], dt.bfloat16, tag="yst", name="yst")
                    for wi, w in enumerate(g):
                        ps = psA.tile([128, F], dt.float32, tag="psA", space="PSUM")
                        nc.tensor.matmul(ps[:], lhsT=xT[:, w * 128:(w + 1) * 128],
                                         rhs=c_W[0][:], start=True, stop=True)
                        stage_y(ps, w, wi, 0, yst)
                    flush_y(g, 0, yst)
                    while nags < NQ and AG_AFTER_GROUP[nags] == gi:
                        ag_quarter(0, nags)
                        nags += 1

            # ---- layers ----
            for layer in range(3):
                nags = 0
                for gi, g in enumerate(groups):
                    m_q = [[gpool.tile([128, cap[q][i], F], dt.bfloat16,
                                       tag=f"m{q}_{i}", name=f"m{q}_{i}")
                            for i in range(2)] for q in range(NQ)]
                    for q in range(NQ):
                        for i in range(2):
                            q0, q1 = gr[gi][q][i]
                            nq = q1 - q0
                            nc.gpsimd.dma_gather(
                                out_ap=m_q[q][i][:, :nq // 128, :],
                                in_ap=y_full[layer][q][:],
                                idxs_ap=c_iq[q][:, q0 // 16:q1 // 16],
                                num_idxs=nq, num_idxs_reg=nq, elem_size=F,
                                queue_num=2 * q + i, single_packet=False)
                    yst = stg.tile([128, G, F], dt.bfloat16, tag="yst", name="yst")
                    ost = stg.tile([128, G, F_OUT], dt.float32, tag="ost", name="ost")
                    for wi, w in enumerate(g):
                        nblk = int(blk_w[w])
                        B0 = int(off_w[w])
                        # one-hot S for the whole window in one op
                        S = sbld.tile([128, nblk, 128], dt.bfloat16, tag="S", name="S")
                        dl_b = (c_dlw[:, B0:B0 + nblk].unsqueeze(2)
                                .broadcast_to([128, nblk, 128]))
                        nc.vector.tensor_tensor(
                            out=S[:, :, :], in0=dl_b,
                            in1=c_iota[:].unsqueeze(1).broadcast_to([128, nblk, 128]),
                            op=Alu.is_equal)
                        # scatter-add via PSUM-accumulated one-hot matmuls
                        agg = psA.tile([128, F], dt.float32, tag="psA", space="PSUM")
                        k = 0
                        for q in range(NQ):
                            for b in range(int(blk[w, q])):
                                gslot = int(off[q, w]) + b * 128
                                sub = 0 if gslot < gr[gi][q][0][1] else 1
                                Bq = (gslot - gr[gi][q][sub][0]) // 128
                                nc.tensor.matmul(
                                    agg[:], lhsT=S[:, k, :],
                                    rhs=m_q[q][sub][:, Bq, :],
                                    start=(k == 0), stop=(k == nblk - 1))
                                k += 1
                        # ---- epilogue: h = dinv*agg + y2b ----
                        h = wpool.tile([128, F], dt.float32, tag="h", name="h")
                        nc.vector.scalar_tensor_tensor(
                            out=h[:], in0=agg[:], scalar=c_dinv[:, w:w + 1],
                            in1=y2b[layer % 2][:, w, :], op0=Alu.mult, op1=Alu.add)
                        if layer < 2:
                            tp = psT.tile([128, 128], dt.float32, tag="tp", space="PSUM")
                            nc.tensor.transpose(tp[:], h[:], c_ident[:])
                            hT = wpool.tile([128, F], dt.float32, tag="hT", name="hT")
                            nc.scalar.copy(hT[:], tp[:])
                            ps2 = ps2p.tile([128, F], dt.float32, tag="ps2", space="PSUM")
                            nc.tensor.matmul(ps2[:], lhsT=hT[:], rhs=c_W[layer + 1][:],
                                             start=True, stop=True)
                            stage_y(ps2, w, wi, layer + 1, yst)
                        else:
                            nc.scalar.activation(ost[:, wi, :], h[:, :F_OUT], Act.Relu)
                    if layer < 2:
                        flush_y(g, layer + 1, yst)
                        while nags < NQ and AG_AFTER_GROUP[nags] == gi:
                            ag_quarter(layer + 1, nags)
                            nags += 1
                    else:
                        w0, w1 = g[0], g[-1] + 1
                        nc.sync.dma_start(
                            t_out[w0 * 128:w1 * 128, :]
                            .rearrange("(t p) f -> p t f", p=128),
                            ost[:, :w1 - w0, :])

    nc.compile()

    # ---- per-core inputs ----
    xT_all = np.zeros((128, N_PAD), np.float32)
    xT_all[:, :N_NODES] = np.asarray(x, np.float32).T
    iota_m = np.broadcast_to(np.arange(128, dtype=np.float32), (128, 128)).astype(BF16)
    in_maps = []
    for c in range(N_CORES):
        rows = slice(c * PER_CORE, (c + 1) * PER_CORE)
        din = dinv_pad[rows].reshape(N_WIN, 128).T.copy()  # [128, N_WIN]
        in_map = {
            "xT_own": np.ascontiguousarray(xT_all[:, rows]),
            "dinv_own": din,
            "dinv2_own": din * din,
            "iota": iota_m.copy(),
            "dl_win": dl_win[c].reshape(-1, 128).T.astype(BF16).copy(),
        }
        for q in range(NQ):
            in_map[f"idx_q{q}"] = _wrap_idx16(idx_q[q][c])
        for i in range(3):
            in_map[f"W{i+1}m"] = Ws[i].copy()
            in_map[f"b{i+1}m"] = np.broadcast_to(bs[i], (128, F)).copy()
        in_maps.append(in_map)

    if sim:
        from concourse.bass_interp import MultiCoreSim
        mcs = MultiCoreSim(nc, num_cores=N_CORES, trace=False,
                           require_finite=False, require_nnan=False)
        for ci, core in enumerate(mcs.cores.values()):
            for k, v in in_maps[ci].items():
                core.tensor(k)[:] = v
        mcs.simulate(check_with_hw=False)
        outs = [np.asarray(core.tensor("h_out"))
                for core in mcs.cores.values()]
        res = None
    else:
        res = bass_utils.run_bass_kernel_spmd(
            nc, in_maps, core_ids=list(range(N_CORES)), trace=trace)
        outs = [r["h_out"] for r in res.results]
    full = np.concatenate(outs, axis=0)[:N_NODES]
    return full, res


def kernel(**inputs) -> np.ndarray:
    edge_index = np.asarray(inputs["edge_index"])
    prep = _preprocess(edge_index)
    out, _ = _build_and_run(inputs, *prep)
    return out
